# revision 4
# baseline (speedup 1.0000x reference)
"""Trainium2 Bass kernel for CTCDecoder-like module.

Reference computes (per batch b, with A = x[:, b, :] of shape (L, D)):
    wx     = A @ Ww^T + Wb
    scores = A @ wx^T                       # (L, L) -- never materialized here
    y      = scores @ A
    logits = y @ Lw^T + Lb
    out    = log_softmax(logits, axis=-1)

Algebraic collapse used by this kernel (exact in real arithmetic):
    scores = A Ww A^T + (A Wb) 1^T
    y      = A Ww (A^T A) + (A Wb)(1^T A) = A (Ww G + Wb c^T) = A H
    logits = A (H Lw^T) + 1 Lb^T = A Mt + 1 Lb^T
with G = A^T A (D x D Gram), c = A^T 1 (column sums), H = Ww G + Wb c^T,
Mt = H Lw^T (D x V).  The (L, L) score matrix is never formed.

Sharding: 8 cores = 4 batches x 2 halves of L.  Each core computes its
batch's Gram redundantly (pure SPMD, no collectives), then its own
2048-token logits + log_softmax.

v2: all matmul operands in bf16 (fp32 "HIGH" mode streams ~2 cycles/row
on TRN2 HW while bf16 streams 1; input DMA also halves).  PSUM stays
fp32 and all softmax math is fp32.  V padded to 1024 with Lw-pad=0 /
Lb-pad=-1e30 so each token tile does one 1024-wide exp/reduce/add.
Column sums ride on Vector+GpSimd accumulators instead of the PE; the
Lb bias is a rank-1 (K=1) matmul so LDWEIGHTS is 1 row, not 128.
"""

import numpy as np
import ml_dtypes

L, B, D, V = 4096, 4, 512, 1000
P = 128
NCORES = 8
LC = L // 2            # tokens per core
NKT = L // P           # 32 k-tiles over full L (Gram)
NLT = LC // P          # 16 l-tiles per core
NDT = D // P           # 4 d-tiles
VP = 1024              # V padded to 2 psum banks
VH = VP // 2           # 512 per half
VS = 640               # vector/scalar split point for the final +shift
NV = 2

BF = ml_dtypes.bfloat16
_ONES_COL = np.ones((P, 1), BF)
_ONES_ROW = np.ones((1, P), BF)
_IDENT = np.eye(P, dtype=BF)

_CACHED_NC = None


def _build_nc():
    import concourse.bass as bass
    import concourse.tile as tile
    import concourse.mybir as mybir
    from concourse import bacc
    from concourse.bass import ds, ts

    f32 = mybir.dt.float32
    bf16 = mybir.dt.bfloat16
    EXP = mybir.ActivationFunctionType.Exp
    LN = mybir.ActivationFunctionType.Ln
    COPY = mybir.ActivationFunctionType.Copy
    IDENT = mybir.ActivationFunctionType.Identity
    X = mybir.AxisListType.X
    SUB = mybir.AluOpType.subtract

    # Serve Exp/Ln from the single act-func table that contains them all
    # (avoids 1283 ns table reloads on ScalarE).
    import concourse.bacc as bacc_mod
    from concourse.hw_specs import get_activation_tables

    def _pinned_tables(arch, _orig=get_activation_tables):
        tables = _orig(arch)
        keep = "natural_log_exp_and_others"
        if keep in tables:
            tables = {
                name: (funcs if name == keep else set())
                for name, funcs in tables.items()
            }
        return tables

    bacc_mod.get_activation_tables = _pinned_tables

    nc = bacc.Bacc("TRN2", target_bir_lowering=False, debug=False)

    a_dram = nc.dram_tensor("a_full", (NKT, P, D), bf16, kind="ExternalInput")
    at_dram = nc.dram_tensor("at", (P, NDT, LC), bf16, kind="ExternalInput")
    wwt_dram = nc.dram_tensor("wwt", (P, NDT, D), bf16, kind="ExternalInput")
    lwt_dram = nc.dram_tensor("lwt", (P, NDT, VP), bf16, kind="ExternalInput")
    wb_dram = nc.dram_tensor("wb", (1, D), bf16, kind="ExternalInput")
    lb_dram = nc.dram_tensor("lb", (1, VP), bf16, kind="ExternalInput")
    onesc_dram = nc.dram_tensor("ones_col", (P, 1), bf16, kind="ExternalInput")
    onesr_dram = nc.dram_tensor("ones_row", (1, P), bf16, kind="ExternalInput")
    ident_dram = nc.dram_tensor("ident", (P, P), bf16, kind="ExternalInput")
    out_dram = nc.dram_tensor("out", (NLT, P, V), f32, kind="ExternalOutput")

    with tile.TileContext(nc) as tc:
        with (
            tc.tile_pool(name="const", bufs=1) as const,
            tc.tile_pool(name="big", bufs=1) as big,
            tc.tile_pool(name="astream", bufs=8) as astream,
            tc.tile_pool(name="escp", bufs=2) as escp,
            tc.tile_pool(name="outp", bufs=4) as outp,
            tc.tile_pool(name="stat", bufs=4) as stat,
            tc.tile_pool(name="ps", bufs=1, space="PSUM") as ps,
        ):
            ident_sb = const.tile([P, P], bf16, name="ident_sb", tag="ident_sb")
            ones_col = const.tile([P, 1], bf16, name="ones_col", tag="ones_col")
            ones_row = const.tile([1, P], bf16, name="ones_row", tag="ones_row")
            wb_sb = const.tile([1, D], bf16, name="wb_sb", tag="wb_sb")
            lb_sb = const.tile([1, VP], bf16, name="lb_sb", tag="lb_sb")
            at_sb = big.tile([P, NDT, LC], bf16, name="at_sb", tag="at_sb")
            wwt_sb = big.tile([P, NDT, D], bf16, name="wwt_sb", tag="wwt_sb")
            lwt_sb = big.tile([P, NDT, VP], bf16, name="lwt_sb", tag="lwt_sb")

            # ---- phase 1: Gram G = A^T A (upper blocks) over full L;
            #      column sums on Vector+GpSimd accumulators in parallel ----
            acc_g = big.tile([P, D], bf16, name="acc_g", tag="acc_g")
            nc.gpsimd.memset(acc_g, 0.0)
            g_ps = [
                ps.tile([P, D], f32, name=f"g_ps{mt}", tag=f"q{mt}")
                for mt in range(NDT)
            ]
            for kt in range(NKT):
                a_t = astream.tile([P, D], bf16, name="a_t", tag="a_t")
                nc.sync.dma_start(a_t, a_dram[kt])
                first, last = kt == 0, kt == NKT - 1
                for mt in range(NDT):
                    n0 = (mt if mt < 3 else 2) * P
                    nc.tensor.matmul(
                        g_ps[mt][:, n0:D], a_t[:, ts(mt, P)], a_t[:, n0:D],
                        start=first, stop=last,
                    )
                nc.gpsimd.tensor_add(acc_g, acc_g, a_t)

            # resident operands stream in while the PE chews on the Gram
            nc.sync.dma_start(wwt_sb, wwt_dram[:])
            nc.sync.dma_start(ident_sb, ident_dram[:])
            nc.sync.dma_start(ones_col, onesc_dram[:])
            nc.sync.dma_start(wb_sb, wb_dram[:])
            nc.sync.dma_start(ones_row, onesr_dram[:])
            nc.sync.dma_start(lb_sb, lb_dram[:])
            nc.sync.dma_start(at_sb, at_dram[:])
            nc.sync.dma_start(lwt_sb, lwt_dram[:])

            # c^T = 1^T A via one rank-reduce matmul over the summed stream
            ct_ps = ps.tile([1, D], f32, name="ct_ps", tag="q0")
            nc.tensor.matmul(ct_ps, ones_col, acc_g, start=True, stop=True)
            c_row = big.tile([1, D], bf16, name="c_row", tag="c_row")
            nc.vector.tensor_copy(c_row, ct_ps)

            g_sb = big.tile([P, NDT, D], bf16, name="g_sb", tag="g_sb")
            for mt in range(NDT):
                n0 = (mt if mt < 3 else 2) * P
                if mt % 2 == 0:
                    nc.vector.tensor_copy(g_sb[:, mt, n0:D], g_ps[mt][:, n0:D])
                else:
                    nc.scalar.activation(g_sb[:, mt, n0:D], g_ps[mt][:, n0:D], COPY)
            # G is symmetric: lower blocks (i,j) are transposes of stored (j,i)
            for idx, (i, j) in enumerate([(1, 0), (2, 0), (3, 0), (2, 1), (3, 1)]):
                tp = ps.tile([P, P], bf16, name=f"tp{i}{j}", tag=f"q{1 + idx % 3}")
                nc.tensor.transpose(tp, g_sb[:, j, ts(i, P)], ident_sb)
                if idx % 2 == 0:
                    nc.vector.tensor_copy(g_sb[:, i, ts(j, P)], tp)
                else:
                    nc.scalar.activation(g_sb[:, i, ts(j, P)], tp, COPY)

            # ---- phase 2: Ht = G Ww^T + c (x) Wb ; Mt = Ht^T Lw^T ----
            ht_sb = big.tile([P, NDT, D], bf16, name="ht_sb", tag="ht_sb")
            for jt in range(NDT):
                hp = ps.tile([P, D], f32, name=f"hp{jt}", tag=f"q{jt}")
                for kt in range(NDT):
                    nc.tensor.matmul(
                        hp, g_sb[:, kt, ts(jt, P)], wwt_sb[:, kt, :],
                        start=(kt == 0), stop=False,
                    )
                nc.tensor.matmul(
                    hp, c_row[:, ts(jt, P)], wb_sb, start=False, stop=True,
                )
                if jt % 2 == 0:
                    nc.vector.tensor_copy(ht_sb[:, jt, :], hp)
                else:
                    nc.scalar.activation(ht_sb[:, jt, :], hp, COPY)

            mt_sb = big.tile([P, NDT, VP], bf16, name="mt_sb", tag="mt_sb")
            for dt in range(NDT):
                for nt in range(NV):
                    mp = ps.tile(
                        [P, VH], f32, name=f"mp{dt}_{nt}",
                        tag=f"q{(dt * NV + nt) % 4}",
                    )
                    for jt in range(NDT):
                        nc.tensor.matmul(
                            mp,
                            ht_sb[:, jt, ts(dt, P)],
                            lwt_sb[:, jt, ds(nt * VH, VH)],
                            start=(jt == 0), stop=(jt == NDT - 1),
                        )
                    if (dt * NV + nt) % 2 == 0:
                        nc.vector.tensor_copy(mt_sb[:, dt, ds(nt * VH, VH)], mp)
                    else:
                        nc.scalar.activation(
                            mt_sb[:, dt, ds(nt * VH, VH)], mp, COPY)

            # ---- phase 3: logits = A Mt + 1 Lb^T, then log_softmax rows ----
            for lt in range(NLT):
                lp = ps.tile([P, VP], f32, name=f"lp{lt}", tag=f"q{lt % 4}")
                for nt in range(NV):
                    half = lp[:, ds(nt * VH, VH)]
                    for kt in range(NDT):
                        nc.tensor.matmul(
                            half,
                            at_sb[:, kt, ts(lt, P)],
                            mt_sb[:, kt, ds(nt * VH, VH)],
                            start=(kt == 0), stop=False,
                        )
                    nc.tensor.matmul(
                        half, ones_row, lb_sb[:, ds(nt * VH, VH)],
                        start=False, stop=True,
                    )

                nmx = stat.tile([P, 1], f32, name="nmx", tag="nmx")
                nc.vector.reduce_max(nmx, lp, axis=X, negate=True)
                esc = escp.tile([P, VP], bf16, name="esc", tag="esc")
                se = stat.tile([P, 1], f32, name="se", tag="se")
                nc.scalar.activation(
                    esc, lp, EXP, bias=nmx, scale=1.0, accum_out=se
                )
                lns = stat.tile([P, 1], f32, name="lns", tag="lns")
                nc.scalar.activation(lns, se, LN)
                shift = stat.tile([P, 1], f32, name="shift", tag="shift")
                nc.vector.tensor_tensor(shift, nmx, lns, op=SUB)

                out_sb = outp.tile([P, VP], f32, name="out_sb", tag="out_sb")
                nc.vector.tensor_scalar_add(out_sb[:, 0:VS], lp[:, 0:VS], shift)
                nc.scalar.activation(
                    out_sb[:, VS:VP], lp[:, VS:VP], IDENT,
                    bias=shift, scale=1.0,
                )
                nc.sync.dma_start(out_dram[lt], out_sb[:, 0:V])

    nc.compile()
    return nc


def _get_nc():
    global _CACHED_NC
    if _CACHED_NC is None:
        _CACHED_NC = _build_nc()
    return _CACHED_NC


def _make_in_maps(x, Ww, Wb, Lw, Lb):
    x = np.asarray(x, dtype=np.float32)
    Ww = np.asarray(Ww, dtype=np.float32)
    Wb = np.asarray(Wb, dtype=np.float32)
    Lw = np.asarray(Lw, dtype=np.float32)
    Lb = np.asarray(Lb, dtype=np.float32)

    wwt = np.ascontiguousarray(
        Ww.T.reshape(NDT, P, D).transpose(1, 0, 2)
    ).astype(BF)  # (P, NDT, D)
    lwt_full = np.zeros((D, VP), np.float32)
    lwt_full[:, :V] = Lw.T
    lwt = np.ascontiguousarray(
        lwt_full.reshape(NDT, P, VP).transpose(1, 0, 2)
    ).astype(BF)  # (P, NDT, VP)
    wb = np.ascontiguousarray(Wb.reshape(1, D)).astype(BF)
    lb = np.full((1, VP), -1e30, np.float32)
    lb[0, :V] = Lb
    lb = lb.astype(BF)

    in_maps = []
    for core in range(NCORES):
        b, h = core // 2, core % 2
        a_b = x[:, b, :]                                       # (L, D) f32
        a_full = np.ascontiguousarray(a_b.reshape(NKT, P, D)).astype(BF)
        at = np.ascontiguousarray(
            a_b[h * LC:(h + 1) * LC, :].T.reshape(NDT, P, LC).transpose(1, 0, 2)
        ).astype(BF)  # (P, NDT, LC)
        in_maps.append({
            "a_full": a_full,
            "at": at,
            "wwt": wwt,
            "lwt": lwt,
            "wb": wb,
            "lb": lb,
            "ones_col": _ONES_COL,
            "ones_row": _ONES_ROW,
            "ident": _IDENT,
        })
    return in_maps


def kernel(x, Ww, Wb, Lw, Lb, _trace=False):
    from concourse.bass_utils import run_bass_kernel_spmd

    nc = _get_nc()
    in_maps = _make_in_maps(x, Ww, Wb, Lw, Lb)
    res = run_bass_kernel_spmd(
        nc, in_maps, core_ids=list(range(NCORES)), trace=_trace
    )
    out = np.empty((L, B, V), np.float32)
    for core in range(NCORES):
        b, h = core // 2, core % 2
        out[h * LC:(h + 1) * LC, b, :] = (
            res.results[core]["out"].reshape(LC, V)
        )
    if _trace:
        kernel._last_results = res
    return out


# revision 5
# speedup vs baseline: 1.2206x; 1.2206x over previous
"""Trainium2 Bass kernel for CTCDecoder-like module.

Reference computes (per batch b, with A = x[:, b, :] of shape (L, D)):
    wx     = A @ Ww^T + Wb
    scores = A @ wx^T                       # (L, L) -- never materialized here
    y      = scores @ A
    logits = y @ Lw^T + Lb
    out    = log_softmax(logits, axis=-1)

Algebraic collapse used by this kernel (exact in real arithmetic):
    scores = A Ww A^T + (A Wb) 1^T
    y      = A Ww (A^T A) + (A Wb)(1^T A) = A (Ww G + Wb c^T) = A H
    logits = A (H Lw^T) + 1 Lb^T = A Mt + 1 Lb^T
with G = A^T A (D x D Gram), c = A^T 1 (column sums), H = Ww G + Wb c^T,
Mt = H Lw^T (D x V).  The (L, L) score matrix is never formed.

Sharding: 8 cores = 4 batches x 2 halves of L.  Each core computes its
batch's Gram redundantly (pure SPMD, no collectives), then its own
2048-token logits + log_softmax.

v2: all matmul operands in bf16 (fp32 "HIGH" mode streams ~2 cycles/row
on TRN2 HW while bf16 streams 1; input DMA also halves).  PSUM stays
fp32 and all softmax math is fp32.  V padded to 1024 with Lw-pad=0 /
Lb-pad=-1e30 so each token tile does one 1024-wide exp/reduce/add.
Column sums ride on Vector+GpSimd accumulators instead of the PE; the
Lb bias is a rank-1 (K=1) matmul so LDWEIGHTS is 1 row, not 128.
"""

import numpy as np
import ml_dtypes

L, B, D, V = 4096, 4, 512, 1000
P = 128
NCORES = 8
LC = L // 2            # tokens per core
NKT = L // P           # 32 k-tiles over full L (Gram)
NLT = LC // P          # 16 l-tiles per core
NDT = D // P           # 4 d-tiles
VP = 1024              # V padded to 2 psum banks
VH = VP // 2           # 512 per half
VS = 640               # vector/scalar split point for the final +shift
NV = 2

BF = ml_dtypes.bfloat16
_ONES_COL = np.ones((P, 1), BF)
_ONES_ROW = np.ones((1, P), BF)
_IDENT = np.eye(P, dtype=BF)

_CACHED_NC = None


def _build_nc():
    import concourse.bass as bass
    import concourse.tile as tile
    import concourse.mybir as mybir
    from concourse import bacc
    from concourse.bass import ds, ts

    f32 = mybir.dt.float32
    bf16 = mybir.dt.bfloat16
    EXP = mybir.ActivationFunctionType.Exp
    LN = mybir.ActivationFunctionType.Ln
    COPY = mybir.ActivationFunctionType.Copy
    IDENT = mybir.ActivationFunctionType.Identity
    X = mybir.AxisListType.X
    SUB = mybir.AluOpType.subtract

    # Serve Exp/Ln from the single act-func table that contains them all
    # (avoids 1283 ns table reloads on ScalarE).
    import concourse.bacc as bacc_mod
    from concourse.hw_specs import get_activation_tables

    def _pinned_tables(arch, _orig=get_activation_tables):
        tables = _orig(arch)
        keep = "natural_log_exp_and_others"
        if keep in tables:
            tables = {
                name: (funcs if name == keep else set())
                for name, funcs in tables.items()
            }
        return tables

    bacc_mod.get_activation_tables = _pinned_tables

    nc = bacc.Bacc("TRN2", target_bir_lowering=False, debug=False)

    a_dram = nc.dram_tensor("a_full", (NKT, P, D), bf16, kind="ExternalInput")
    at_dram = nc.dram_tensor("at", (P, NDT, LC), bf16, kind="ExternalInput")
    wwt_dram = nc.dram_tensor("wwt", (P, NDT, D), bf16, kind="ExternalInput")
    lwt_dram = nc.dram_tensor("lwt", (P, NDT, VP), bf16, kind="ExternalInput")
    wb_dram = nc.dram_tensor("wb", (1, D), bf16, kind="ExternalInput")
    lb_dram = nc.dram_tensor("lb", (1, VP), bf16, kind="ExternalInput")
    onesc_dram = nc.dram_tensor("ones_col", (P, 1), bf16, kind="ExternalInput")
    onesr_dram = nc.dram_tensor("ones_row", (1, P), bf16, kind="ExternalInput")
    ident_dram = nc.dram_tensor("ident", (P, P), bf16, kind="ExternalInput")
    out_dram = nc.dram_tensor("out", (NLT, P, V), f32, kind="ExternalOutput")

    with tile.TileContext(nc) as tc:
        with (
            tc.tile_pool(name="const", bufs=1) as const,
            tc.tile_pool(name="big", bufs=1) as big,
            tc.tile_pool(name="astream", bufs=8) as astream,
            tc.tile_pool(name="escp", bufs=2) as escp,
            tc.tile_pool(name="outp", bufs=4) as outp,
            tc.tile_pool(name="stat", bufs=4) as stat,
            tc.tile_pool(name="ps", bufs=1, space="PSUM") as ps,
        ):
            ident_sb = const.tile([P, P], bf16, name="ident_sb", tag="ident_sb")
            ones_col = const.tile([P, 1], bf16, name="ones_col", tag="ones_col")
            ones_row = const.tile([1, P], bf16, name="ones_row", tag="ones_row")
            wb_sb = const.tile([1, D], bf16, name="wb_sb", tag="wb_sb")
            lb_sb = const.tile([1, VP], bf16, name="lb_sb", tag="lb_sb")
            at_sb = big.tile([P, NDT, LC], bf16, name="at_sb", tag="at_sb")
            wwt_sb = big.tile([P, NDT, D], bf16, name="wwt_sb", tag="wwt_sb")
            lwt_sb = big.tile([P, NDT, VP], bf16, name="lwt_sb", tag="lwt_sb")

            # ---- phase 1: Gram G = A^T A (upper blocks) over full L;
            #      column sums on Vector+GpSimd accumulators in parallel ----
            acc_g = big.tile([P, D], bf16, name="acc_g", tag="acc_g")
            nc.vector.memset(acc_g, 0.0)
            # spin the PE p-state up while the first a-tiles are in flight
            warm_in = big.tile([P, D], bf16, name="warm_in", tag="warm_in")
            nc.gpsimd.memset(warm_in, 0.0)
            for w in range(12):
                wp = ps.tile([P, D], f32, name=f"wp{w}", tag="q3")
                nc.tensor.matmul(
                    wp, warm_in[:, 0:P], warm_in, start=True, stop=True,
                    skip_group_check=True,
                )
            g_ps = [
                ps.tile([P, D], f32, name=f"g_ps{mt}", tag=f"q{mt}")
                for mt in range(NDT)
            ]
            for kt in range(NKT):
                a_t = astream.tile([P, D], bf16, name="a_t", tag="a_t")
                nc.sync.dma_start(a_t, a_dram[kt])
                first, last = kt == 0, kt == NKT - 1
                for mt in range(NDT):
                    n0 = (mt if mt < 3 else 2) * P
                    n1 = 3 * P if mt == 2 else D
                    nc.tensor.matmul(
                        g_ps[mt][:, n0:n1], a_t[:, ts(mt, P)], a_t[:, n0:n1],
                        start=first, stop=last,
                    )
                nc.vector.tensor_add(acc_g, acc_g, a_t)

            # resident operands stream in while the PE chews on the Gram
            nc.sync.dma_start(wwt_sb, wwt_dram[:])
            nc.sync.dma_start(ident_sb, ident_dram[:])
            nc.sync.dma_start(ones_col, onesc_dram[:])
            nc.sync.dma_start(wb_sb, wb_dram[:])
            nc.sync.dma_start(ones_row, onesr_dram[:])
            nc.sync.dma_start(lb_sb, lb_dram[:])
            nc.sync.dma_start(at_sb, at_dram[:])
            nc.sync.dma_start(lwt_sb, lwt_dram[:])

            # c^T = 1^T A via one rank-reduce matmul over the summed stream
            ct_ps = ps.tile([1, D], f32, name="ct_ps", tag="q0")
            nc.tensor.matmul(ct_ps, ones_col, acc_g, start=True, stop=True)
            c_row = big.tile([1, D], bf16, name="c_row", tag="c_row")
            nc.vector.tensor_copy(c_row, ct_ps)

            g_sb = big.tile([P, NDT, D], bf16, name="g_sb", tag="g_sb")
            for mt in range(NDT):
                n0 = (mt if mt < 3 else 2) * P
                n1 = 3 * P if mt == 2 else D
                if mt % 2 == 0:
                    nc.vector.tensor_copy(g_sb[:, mt, n0:n1], g_ps[mt][:, n0:n1])
                else:
                    nc.scalar.activation(g_sb[:, mt, n0:n1], g_ps[mt][:, n0:n1], COPY)
            # G is symmetric: lower blocks (i,j) are transposes of stored (j,i)
            for idx, (i, j) in enumerate(
                [(1, 0), (2, 0), (3, 0), (2, 1), (3, 1), (2, 3)]
            ):
                tp = ps.tile([P, P], bf16, name=f"tp{i}{j}", tag=f"q{1 + idx % 3}")
                nc.tensor.transpose(tp, g_sb[:, j, ts(i, P)], ident_sb)
                if idx % 2 == 0:
                    nc.vector.tensor_copy(g_sb[:, i, ts(j, P)], tp)
                else:
                    nc.scalar.activation(g_sb[:, i, ts(j, P)], tp, COPY)

            # ---- phase 2: Ht = G Ww^T + c (x) Wb ; Mt = Ht^T Lw^T ----
            ht_sb = big.tile([P, NDT, D], bf16, name="ht_sb", tag="ht_sb")
            for jt in range(NDT):
                hp = ps.tile([P, D], f32, name=f"hp{jt}", tag=f"q{jt}")
                for kt in range(NDT):
                    nc.tensor.matmul(
                        hp, g_sb[:, kt, ts(jt, P)], wwt_sb[:, kt, :],
                        start=(kt == 0), stop=False,
                    )
                nc.tensor.matmul(
                    hp, c_row[:, ts(jt, P)], wb_sb, start=False, stop=True,
                )
                if jt % 2 == 0:
                    nc.vector.tensor_copy(ht_sb[:, jt, :], hp)
                else:
                    nc.scalar.activation(ht_sb[:, jt, :], hp, COPY)

            mt_sb = big.tile([P, NDT, VP], bf16, name="mt_sb", tag="mt_sb")
            for dt in range(NDT):
                for nt in range(NV):
                    mp = ps.tile(
                        [P, VH], f32, name=f"mp{dt}_{nt}",
                        tag=f"q{(dt * NV + nt) % 4}",
                    )
                    for jt in range(NDT):
                        nc.tensor.matmul(
                            mp,
                            ht_sb[:, jt, ts(dt, P)],
                            lwt_sb[:, jt, ds(nt * VH, VH)],
                            start=(jt == 0), stop=(jt == NDT - 1),
                        )
                    if (dt * NV + nt) % 2 == 0:
                        nc.vector.tensor_copy(mt_sb[:, dt, ds(nt * VH, VH)], mp)
                    else:
                        nc.scalar.activation(
                            mt_sb[:, dt, ds(nt * VH, VH)], mp, COPY)

            # ---- phase 3: logits = A Mt + 1 Lb^T, then log_softmax rows ----
            for lt in range(NLT):
                lp = ps.tile([P, VP], f32, name=f"lp{lt}", tag=f"q{lt % 4}")
                for kt in range(NDT):
                    for nt in range(NV):
                        nc.tensor.matmul(
                            lp[:, ds(nt * VH, VH)],
                            at_sb[:, kt, ts(lt, P)],
                            mt_sb[:, kt, ds(nt * VH, VH)],
                            start=(kt == 0), stop=False,
                        )
                for nt in range(NV):
                    nc.tensor.matmul(
                        lp[:, ds(nt * VH, VH)], ones_row,
                        lb_sb[:, ds(nt * VH, VH)],
                        start=False, stop=True,
                    )

                nmx = stat.tile([P, 1], f32, name="nmx", tag="nmx")
                nc.vector.reduce_max(nmx, lp, axis=X, negate=True)
                esc = escp.tile([P, VP], bf16, name="esc", tag="esc")
                se = stat.tile([P, 1], f32, name="se", tag="se")
                nc.scalar.activation(
                    esc, lp, EXP, bias=nmx, scale=1.0, accum_out=se
                )
                lns = stat.tile([P, 1], f32, name="lns", tag="lns")
                nc.scalar.activation(lns, se, LN)
                shift = stat.tile([P, 1], f32, name="shift", tag="shift")
                nc.vector.tensor_tensor(shift, nmx, lns, op=SUB)

                out_sb = outp.tile([P, VP], f32, name="out_sb", tag="out_sb")
                nc.vector.tensor_scalar_add(out_sb[:, 0:VS], lp[:, 0:VS], shift)
                nc.scalar.activation(
                    out_sb[:, VS:VP], lp[:, VS:VP], IDENT,
                    bias=shift, scale=1.0,
                )
                nc.sync.dma_start(out_dram[lt], out_sb[:, 0:V])

    nc.compile()
    return nc


def _get_nc():
    global _CACHED_NC
    if _CACHED_NC is None:
        _CACHED_NC = _build_nc()
    return _CACHED_NC


def _make_in_maps(x, Ww, Wb, Lw, Lb):
    x = np.asarray(x, dtype=np.float32)
    Ww = np.asarray(Ww, dtype=np.float32)
    Wb = np.asarray(Wb, dtype=np.float32)
    Lw = np.asarray(Lw, dtype=np.float32)
    Lb = np.asarray(Lb, dtype=np.float32)

    wwt = np.ascontiguousarray(
        Ww.T.reshape(NDT, P, D).transpose(1, 0, 2)
    ).astype(BF)  # (P, NDT, D)
    lwt_full = np.zeros((D, VP), np.float32)
    lwt_full[:, :V] = Lw.T
    lwt = np.ascontiguousarray(
        lwt_full.reshape(NDT, P, VP).transpose(1, 0, 2)
    ).astype(BF)  # (P, NDT, VP)
    wb = np.ascontiguousarray(Wb.reshape(1, D)).astype(BF)
    lb = np.full((1, VP), -1e30, np.float32)
    lb[0, :V] = Lb
    lb = lb.astype(BF)

    in_maps = []
    for core in range(NCORES):
        b, h = core // 2, core % 2
        a_b = x[:, b, :]                                       # (L, D) f32
        a_full = np.ascontiguousarray(a_b.reshape(NKT, P, D)).astype(BF)
        at = np.ascontiguousarray(
            a_b[h * LC:(h + 1) * LC, :].T.reshape(NDT, P, LC).transpose(1, 0, 2)
        ).astype(BF)  # (P, NDT, LC)
        in_maps.append({
            "a_full": a_full,
            "at": at,
            "wwt": wwt,
            "lwt": lwt,
            "wb": wb,
            "lb": lb,
            "ones_col": _ONES_COL,
            "ones_row": _ONES_ROW,
            "ident": _IDENT,
        })
    return in_maps


def kernel(x, Ww, Wb, Lw, Lb, _trace=False):
    from concourse.bass_utils import run_bass_kernel_spmd

    nc = _get_nc()
    in_maps = _make_in_maps(x, Ww, Wb, Lw, Lb)
    res = run_bass_kernel_spmd(
        nc, in_maps, core_ids=list(range(NCORES)), trace=_trace
    )
    out = np.empty((L, B, V), np.float32)
    for core in range(NCORES):
        b, h = core // 2, core % 2
        out[h * LC:(h + 1) * LC, b, :] = (
            res.results[core]["out"].reshape(LC, V)
        )
    if _trace:
        kernel._last_results = res
    return out
